# revision 11
# baseline (speedup 1.0000x reference)
"""BitTransformerEncoderLayer on 8 TRN2 NeuronCores.

Strategy: pure data parallelism over batch (B=8 == n_cores). Each core runs the
full layer for one batch element; no collectives. BitLinear matmuls run as exact
integer arithmetic in bf16 (ternary weights quantized on host; activations
rounded to ints <=127 on device via the fp32 magic-constant trick), with the
fp32 scales folded into PSUM eviction.

v2 over the first working version:
- all 128x128 transposes (x2T, vT, x2qT) moved from the PE to DMA xbar
  transposes (dma_start_transpose)
- softmax denominator: DVE-sum over key tiles + one ones-matmul per (head, sc)
  instead of 8 ones-matmuls
- norm1_w folded into in_proj weights on host
- phase 3 runs out_proj in two 4-token-tile groups so the rmsnorm2/quant chain
  of group A hides under group B's matmuls; ff1 starts on the first token half
- h2 (ff1 output) stays in SBUF as bf16 and is quantized in place for ff2
  (no DRAM spill); the x+b2 residual spills to DRAM as bf16
- DMA queue separation: weight streams on sync, spills/transposes on scalar

kernel(**inputs) takes the FULL unsharded inputs and returns the FULL output.
"""

import numpy as np

P = 128
EPS = 1e-8
MAGIC = 12582912.0  # 1.5 * 2**23: fp32 add/sub rounds to nearest-even integer
NCORES = 8

B_FULL, S_FULL, D_FULL, H_FULL, DFF_FULL = 8, 1024, 2048, 16, 8192

_CACHE = {}


# ---------------------------------------------------------------- host prep

def _quant_w(w):
    scale = np.maximum(np.mean(np.abs(w), dtype=np.float32), np.float32(1e-5))
    q = np.clip(np.round(w / scale), -1.0, 1.0).astype(np.float32)
    return q, float(scale)


def _lhsT_blocks(w):
    """w [M, K] -> [M/P, P(k), K/P, P(m)]; [mo, :, ko, :] = w-block(mo, ko).T"""
    M, K = w.shape
    t = w.reshape(M // P, P, K // P, P)  # [mo, pm, ko, pk]
    return np.ascontiguousarray(t.transpose(0, 3, 2, 1))


def _rhs_chunks(w, nch):
    """w [N, K] -> [N/nch, K/P, P, nch]; [no, ko, p, j] = w[no*nch+j, ko*P+p]"""
    N, K = w.shape
    t = w.reshape(N // nch, nch, K // P, P)  # [no, j, ko, p]
    return np.ascontiguousarray(t.transpose(0, 2, 3, 1))


def _per_part(v):
    """[M] -> [P, M/P]; out[p, mo] = v[mo*P + p]"""
    return np.ascontiguousarray(v.reshape(-1, P).T)


def _bcast_row(v):
    return np.ascontiguousarray(np.broadcast_to(v[None, :], (P, v.shape[0])))


def _prep_arrays(inputs, S, D, H, DFF):
    import ml_dtypes

    bf16 = ml_dtypes.bfloat16
    f32 = np.float32
    g = lambda k: np.asarray(inputs[k], dtype=f32)

    w1q, ws1 = _quant_w(g("ff1_w"))   # [DFF, D]
    w2q, ws2 = _quant_w(g("ff2_w"))   # [D, DFF]
    ncd = min(512, D)

    w_in_eff = g("in_proj_w") * g("norm1_w")[None, :]  # fold rmsnorm1 weight

    arrays = {
        "w_in_blk": _lhsT_blocks(w_in_eff).astype(bf16),        # [3D/P, P, D/P, P]
        "wo_chunk": _rhs_chunks(g("out_proj_w"), ncd).astype(bf16),  # [D/ncd, D/P, P, ncd]
        "w1_blk": _lhsT_blocks(w1q).astype(bf16),                # [DFF/P, P, D/P, P]
        "w2_chunk": _rhs_chunks(w2q, ncd).astype(bf16),          # [D/ncd, DFF/P, P, ncd]
        "bias_in": _per_part(g("in_proj_b")).astype(f32),        # [P, 3D/P]
        "b1_t": _per_part(g("ff1_b")).astype(f32),               # [P, DFF/P]
        "alpha_t": _per_part(g("alpha")).astype(f32),
        "ab1_t": _per_part((g("alpha") * g("ff1_b")).astype(f32)),
        "gamma_t": _per_part((1.0 / (g("beta") + np.float32(1e-9))).astype(f32)),
        "n2w_bc": _bcast_row(g("norm2_w")).astype(f32),          # [P, D]
        "b2_bc": _bcast_row(g("ff2_b")).astype(f32),
        "ob_bc": _bcast_row(g("out_proj_b")).astype(f32),
    }
    return arrays, ws1, ws2


# ---------------------------------------------------------------- device program

def build_program(nc, *, S, D, H, DFF, ws1, ws2):
    import concourse.mybir as mybir
    import concourse.tile as tile
    from concourse.bass import ts
    from concourse.masks import make_identity

    dt = mybir.dt
    AF = mybir.ActivationFunctionType
    OP = mybir.AluOpType

    DH = D // H
    assert DH == P, "layout assumes head dim == 128"
    ST = S // P           # token tiles
    KD = D // P           # D contraction tiles
    KF = DFF // P         # DFF contraction tiles / ff1 out tiles
    NCD = min(512, D)     # fo chunk for out_proj/ff2 (psum-bank sized)
    NOD = D // NCD
    NCS = min(512, S)     # s chunk
    NOS = S // NCS
    MTG = ST // 2         # token tiles per out_proj group
    inv_sqrt_dh = float(1.0 / np.sqrt(DH))

    # ---- DRAM I/O ----
    src_d = nc.dram_tensor("src", [S, D], dt.float32, kind="ExternalInput")
    srcob_d = nc.dram_tensor("srcob", [S, D], dt.float32, kind="ExternalInput")  # src + out_proj_b
    w_in_d = nc.dram_tensor("w_in_blk", [3 * KD, P, KD, P], dt.bfloat16, kind="ExternalInput")
    wo_d = nc.dram_tensor("wo_chunk", [NOD, KD, P, NCD], dt.bfloat16, kind="ExternalInput")
    w1_d = nc.dram_tensor("w1_blk", [KF, P, KD, P], dt.bfloat16, kind="ExternalInput")
    w2_d = nc.dram_tensor("w2_chunk", [NOD, KF, P, NCD], dt.bfloat16, kind="ExternalInput")
    bin_d = nc.dram_tensor("bias_in", [P, 3 * KD], dt.float32, kind="ExternalInput")
    b1_d = nc.dram_tensor("b1_t", [P, KF], dt.float32, kind="ExternalInput")
    alpha_d = nc.dram_tensor("alpha_t", [P, KF], dt.float32, kind="ExternalInput")
    ab1_d = nc.dram_tensor("ab1_t", [P, KF], dt.float32, kind="ExternalInput")
    gam_d = nc.dram_tensor("gamma_t", [P, KF], dt.float32, kind="ExternalInput")
    n2w_d = nc.dram_tensor("n2w_bc", [P, D], dt.float32, kind="ExternalInput")
    b2_d = nc.dram_tensor("b2_bc", [P, D], dt.float32, kind="ExternalInput")
    out_d = nc.dram_tensor("out", [S, D], dt.float32, kind="ExternalOutput")
    # internal DRAM spill: x + b2 (residual for ff2 eviction), bf16
    xb2_d = nc.dram_tensor("xb2_spill", [ST, P, D], dt.bfloat16)

    with tile.TileContext(nc) as tc:
        # ---------- persistent constants ----------
        cp = tc.alloc_tile_pool(name="consts", bufs=1)
        identf = cp.tile([P, P], dt.float32)
        make_identity(nc, identf)
        ones_kb = cp.tile([P, 1], dt.bfloat16)
        nc.any.memset(ones_kb[:], 1.0)
        ones_1 = cp.tile([1, P], dt.float32)
        nc.any.memset(ones_1[:], 1.0)
        magic_col = cp.tile([P, 1], dt.float32)
        nc.any.memset(magic_col[:], MAGIC)
        bin_sb = cp.tile([P, 3 * KD], dt.float32)
        nc.sync.dma_start(out=bin_sb[:], in_=bin_d.ap())
        b1_sb = cp.tile([P, KF], dt.float32)
        nc.sync.dma_start(out=b1_sb[:], in_=b1_d.ap())
        alpha_sb = cp.tile([P, KF], dt.float32)
        nc.sync.dma_start(out=alpha_sb[:], in_=alpha_d.ap())
        ab1_sb = cp.tile([P, KF], dt.float32)
        nc.sync.dma_start(out=ab1_sb[:], in_=ab1_d.ap())
        gam_sb = cp.tile([P, KF], dt.float32)
        nc.sync.dma_start(out=gam_sb[:], in_=gam_d.ap())
        c2_tok = cp.tile([P, ST], dt.float32)   # filled in phase 4
        sc2_bc = cp.tile([P, S], dt.float32)    # filled in phase 4

        # ================= phase 1: rmsnorm1 (n1w folded) + DMA transpose =====
        xp = tc.alloc_tile_pool(name="x2T_pool", bufs=1)
        x2T = xp.tile([P, KD, S], dt.bfloat16)

        p1 = tc.alloc_tile_pool(name="p1", bufs=2)
        for mt in range(ST):
            xt = p1.tile([P, D], dt.float32, tag="xt", bufs=3)
            nc.sync.dma_start(out=xt[:], in_=src_d.ap()[ts(mt, P), :])
            sq = p1.tile([P, D], dt.float32, tag="sq", bufs=2)
            ss = p1.tile([P, 1], dt.float32, tag="ss")
            nc.scalar.activation(sq[:], xt[:], AF.Square, accum_out=ss[:])
            ms = p1.tile([P, 1], dt.float32, tag="ms")
            nc.vector.tensor_scalar(ms[:], ss[:], 1.0 / D, EPS, op0=OP.mult, op1=OP.add)
            rt = p1.tile([P, 1], dt.float32, tag="rt")
            nc.scalar.activation(rt[:], ms[:], AF.Sqrt)
            rs = p1.tile([P, 1], dt.float32, tag="rs")
            nc.vector.reciprocal(rs[:], rt[:])
            x2 = p1.tile([P, D], dt.bfloat16, tag="x2", bufs=3)
            nc.scalar.activation(x2[:], xt[:], AF.Copy, scale=rs[:])
            nc.scalar.dma_start_transpose(out=x2T[:, :, ts(mt, P)], in_=x2[:])
        p1.release()

        # ================= phase 2: fused in_proj + attention =================
        op_ = tc.alloc_tile_pool(name="oT_pool", bufs=1, side="right")
        oT_all = op_.tile([P, KD, S], dt.bfloat16)

        p2w = tc.alloc_tile_pool(name="p2w", bufs=2)
        p2 = tc.alloc_tile_pool(name="p2", bufs=2)
        p2a = tc.alloc_tile_pool(name="p2a", bufs=2, space="PSUM")
        p2b = tc.alloc_tile_pool(name="p2b", bufs=2, space="PSUM")
        p2c = tc.alloc_tile_pool(name="p2c", bufs=1, space="PSUM")

        def attn_tail(h, es, vT):
            # es[sc] = sum over tt of expT (DVE); denominator = ones-matmul over
            # partitions, broadcast back via a second matmul. One head late so
            # the PE never waits on ACT's exp.
            for sc in range(NOS):
                esb = p2.tile([P, NCS], dt.bfloat16, tag="esb", name=f"esb_{h}_{sc}")
                nc.vector.tensor_copy(esb[:], es[sc][:])
                psr = p2c.tile([1, NCS], dt.float32, tag="den", name=f"psr_{h}_{sc}")
                nc.tensor.matmul(psr[:], ones_kb[:], esb[:], start=True, stop=True)
                srow = p2.tile([1, NCS], dt.float32, tag="srow", name=f"srow_{h}_{sc}")
                nc.scalar.activation(srow[:], psr[:], AF.Copy)
                psb = p2c.tile([P, NCS], dt.float32, tag="den", name=f"psb_{h}_{sc}")
                nc.tensor.matmul(psb[:], ones_1[:], srow[:], start=True, stop=True)
                rb = p2.tile([P, NCS], dt.float32, tag="rb", name=f"rb_{h}_{sc}")
                nc.vector.reciprocal(rb[:], psb[:])
                po = p2a.tile([P, NCS], dt.float32, tag="po", bufs=2, name=f"po_{h}_{sc}")
                for tt in range(ST):
                    nc.tensor.matmul(po[:], vT[:, tt, :], expT_of[h][:, tt, ts(sc, NCS)],
                                     start=(tt == 0), stop=(tt == ST - 1))
                nc.vector.tensor_tensor(oT_all[:, h, ts(sc, NCS)], po[:], rb[:], OP.mult)

        expT_of = {}
        prev = None
        for h in range(H):
            qkv = []
            for j, mo in ((0, h), (1, KD + h), (2, 2 * KD + h)):
                wblk = p2w.tile([P, KD, P], dt.bfloat16, tag="wblk", bufs=3)
                nc.sync.dma_start(out=wblk[:], in_=w_in_d.ap()[mo])
                dest = p2.tile([P, S], dt.bfloat16, tag=f"qkv{j}", name=f"qkv{j}_{h}")
                for sc in range(NOS):
                    ps = p2a.tile([P, NCS], dt.float32, tag="mmps", bufs=2,
                                  name=f"qkvps_{h}_{j}_{sc}")
                    for ko in range(KD):
                        nc.tensor.matmul(ps[:], wblk[:, ko, :], x2T[:, ko, ts(sc, NCS)],
                                         start=(ko == 0), stop=(ko == KD - 1))
                    nc.scalar.activation(dest[:, ts(sc, NCS)], ps[:], AF.Identity,
                                         bias=bin_sb[:, mo:mo + 1])
                qkv.append(dest)
            q, k, v = qkv
            # scores^T -> exp (no max-subtract; scores are O(5)); accumulate
            # the tt-sum for the denominator on DVE as chunks appear.
            expT = p2.tile([P, ST, S], dt.bfloat16, tag="expT", name=f"expT_{h}")
            expT_of[h] = expT
            es = [p2.tile([P, NCS], dt.float32, tag=f"es{sc}", name=f"es_{h}_{sc}")
                  for sc in range(NOS)]
            for tt in range(ST):
                for sc in range(NOS):
                    ps = p2b.tile([P, NCS], dt.float32, tag="scps", name=f"scps_{h}_{tt}_{sc}")
                    nc.tensor.matmul(ps[:], k[:, ts(tt, P)], q[:, ts(sc, NCS)],
                                     start=True, stop=True)
                    nc.scalar.activation(expT[:, tt, ts(sc, NCS)], ps[:], AF.Exp,
                                         scale=inv_sqrt_dh)
                    if tt == 0:
                        nc.vector.tensor_copy(es[sc][:], expT[:, 0, ts(sc, NCS)])
                    else:
                        nc.vector.tensor_tensor(es[sc][:], es[sc][:],
                                                expT[:, tt, ts(sc, NCS)], OP.add)
            # v^T via DMA xbar transpose
            vT = p2.tile([P, ST, P], dt.bfloat16, tag="vT", name=f"vT_{h}")
            nc.scalar.dma_start_transpose(out=vT[:, :, :], in_=v[:])
            if prev is not None:
                attn_tail(*prev)
            prev = (h, es, vT)
        attn_tail(*prev)
        p2c.release()
        p2b.release()
        p2a.release()
        p2.release()
        p2w.release()
        xp.release()  # x2T dead

        # ===== phase 3: out_proj + residual + rmsnorm2 + quant, 2 mt-groups =====
        # Group B's matmuls (and ff1's first tokens) hide group A's
        # rmsnorm/quant chain. wo streamed once per group.
        qp = tc.alloc_tile_pool(name="x2qT_pool", bufs=1)
        x2qT = qp.tile([P, KD, S], dt.bfloat16)
        c1_bc = qp.tile([P, S], dt.float32)

        p3c = tc.alloc_tile_pool(name="p3c", bufs=1)
        p3x = tc.alloc_tile_pool(name="p3x", bufs=1)
        p3 = tc.alloc_tile_pool(name="p3", bufs=3)
        p3q = tc.alloc_tile_pool(name="p3q", bufs=2)
        p3ps = tc.alloc_tile_pool(name="p3ps", bufs=1, space="PSUM")
        n2w_sb = p3c.tile([P, D], dt.float32)
        nc.sync.dma_start(out=n2w_sb[:], in_=n2w_d.ap())
        b2_sb = p3c.tile([P, D], dt.float32)
        nc.sync.dma_start(out=b2_sb[:], in_=b2_d.ap())
        c1row = p3c.tile([1, S], dt.float32)

        xg = {}

        def quant_chain(mt, ssl):
            xt = xg[mt]
            ss = p3q.tile([P, 1], dt.float32, tag="ss", bufs=4, name=f"ssq_{mt}")
            nc.vector.tensor_reduce(ss[:], ssl[:], axis=mybir.AxisListType.X, op=OP.add)
            ms = p3q.tile([P, 1], dt.float32, tag="ms", bufs=4)
            nc.vector.tensor_scalar(ms[:], ss[:], 1.0 / D, EPS, op0=OP.mult, op1=OP.add)
            rt = p3q.tile([P, 1], dt.float32, tag="rt", bufs=4)
            nc.scalar.activation(rt[:], ms[:], AF.Sqrt)
            rs = p3q.tile([P, 1], dt.float32, tag="rs", bufs=4)
            nc.vector.reciprocal(rs[:], rt[:])
            x2 = p3q.tile([P, D], dt.float32, tag="x2", bufs=2)
            nc.vector.scalar_tensor_tensor(x2[:], xt[:], rs[:], n2w_sb[:],
                                           op0=OP.mult, op1=OP.mult)
            mx = p3q.tile([P, 1], dt.float32, tag="mx", bufs=4)
            nc.vector.tensor_reduce(mx[:], x2[:], axis=mybir.AxisListType.X,
                                    op=OP.max, apply_absolute_value=True)
            mcl = p3q.tile([P, 1], dt.float32, tag="mcl", bufs=4)
            nc.vector.tensor_scalar(mcl[:], mx[:], 1e-5, None, op0=OP.max)
            rc = p3q.tile([P, 1], dt.float32, tag="rc", bufs=4)
            nc.vector.reciprocal(rc[:], mcl[:])
            sc1 = p3q.tile([P, 1], dt.float32, tag="sc1", bufs=4)
            nc.vector.tensor_scalar(sc1[:], rc[:], 127.0, None, op0=OP.mult)
            c1c = p3q.tile([P, 1], dt.float32, tag="c1c", bufs=4)
            nc.vector.tensor_scalar(c1c[:], mcl[:], ws1 / 127.0, None, op0=OP.mult)
            nc.scalar.dma_start(out=c1row[0:1, ts(mt, P)], in_=c1c[:])
            # xq = round(x2 * sc1): exact ints in bf16 (ACT does mult+magic)
            t1 = p3q.tile([P, D], dt.float32, tag="t1", bufs=1)
            nc.scalar.activation(t1[:], x2[:], AF.Identity, scale=sc1[:], bias=magic_col[:])
            xq = p3q.tile([P, D], dt.bfloat16, tag="xq", bufs=2)
            nc.vector.tensor_scalar(xq[:], t1[:], MAGIC, None, op0=OP.subtract)
            nc.scalar.dma_start_transpose(out=x2qT[:, :, ts(mt, P)], in_=xq[:])

        for g in range(2):
            mts = range(g * MTG, (g + 1) * MTG)
            for mt in mts:  # seed x tiles with the residual (src + out_proj_b)
                xg[mt] = p3x.tile([P, D], dt.float32, tag=f"xg{mt % MTG}", bufs=1,
                                  name=f"xg_{mt}")
                nc.scalar.dma_start(out=xg[mt][:], in_=srcob_d.ap()[ts(mt, P), :])
            psy = {mt: p3ps.tile([P, NCD], dt.float32, tag=f"y{mt % MTG}", bufs=2,
                                 name=f"psy3_{g}_{mt}")
                   for mt in mts}
            ssl = {mt: p3q.tile([P, NOD], dt.float32, tag=f"ssl{mt % MTG}", bufs=2,
                                name=f"ssl_{g}_{mt}")
                   for mt in mts}
            for no in range(NOD):
                for ko in range(KD):
                    wch = p3.tile([P, NCD], dt.bfloat16, tag="wch", bufs=4)
                    nc.sync.dma_start(out=wch[:], in_=wo_d.ap()[no, ko])
                    for mt in mts:
                        nc.tensor.matmul(psy[mt][:], oT_all[:, ko, ts(mt, P)], wch[:],
                                         start=(ko == 0), stop=(ko == KD - 1))
                ch = ts(no, NCD)
                for mt in mts:
                    nc.vector.tensor_tensor(xg[mt][:, ch], xg[mt][:, ch], psy[mt][:],
                                            OP.add)
                    if no == NOD - 1:
                        continue
                    psy[mt] = p3ps.tile([P, NCD], dt.float32, tag=f"y{mt % MTG}",
                                        bufs=2, name=f"psy3_{g}_{mt}_{no + 1}")
                for mt in mts:
                    # spill x + b2 (bf16) off the critical chain
                    xo = p3.tile([P, NCD], dt.bfloat16, tag="xo", bufs=4,
                                 name=f"xo_{g}_{no}_{mt}")
                    nc.vector.tensor_tensor(xo[:], xg[mt][:, ch], b2_sb[:, ch], OP.add)
                    nc.scalar.dma_start(out=xb2_d.ap()[mt][:, ch], in_=xo[:])
                    sqt = p3.tile([P, NCD], dt.float32, tag="sqt", bufs=4,
                                  name=f"sqt_{g}_{no}_{mt}")
                    nc.scalar.activation(sqt[:], xg[mt][:, ch], AF.Square,
                                         accum_out=ssl[mt][:, no:no + 1])
            for mt in mts:
                quant_chain(mt, ssl[mt])
            # c1 broadcast for this half's s-chunk
            c1ps = p3ps.tile([P, NCS], dt.float32, tag="y0", bufs=2, name=f"c1ps_{g}")
            nc.tensor.matmul(c1ps[:], ones_1[:], c1row[:, ts(g, NCS)],
                             start=True, stop=True)
            nc.vector.tensor_copy(c1_bc[:, ts(g, NCS)], c1ps[:])
        p3ps.release()
        p3q.release()
        p3.release()
        p3x.release()
        p3c.release()
        op_.release()  # oT_all dead

        # ================= phase 4: ff1 + snake -> h2 (bf16, SBUF) + absmax ====
        hp = tc.alloc_tile_pool(name="h2_pool", bufs=1, side="right")
        h2sb = hp.tile([P, KF, S], dt.bfloat16)

        p4 = tc.alloc_tile_pool(name="p4", bufs=2)
        p4m = tc.alloc_tile_pool(name="p4m", bufs=1)
        p4ps = tc.alloc_tile_pool(name="p4ps", bufs=3, space="PSUM")
        M_acc = p4m.tile([P, S], dt.float32)
        nc.any.memset(M_acc[:], 0.0)

        def ff1_unit(mo, sc, wblk):
            ph = p4ps.tile([P, NCS], dt.float32, tag="ph", name=f"ph_{mo}_{sc}")
            for ko in range(KD):
                nc.tensor.matmul(ph[:], wblk[:, ko, :], x2qT[:, ko, ts(sc, NCS)],
                                 start=(ko == 0), stop=(ko == KD - 1))
            ch = ts(sc, NCS)
            t_ = p4.tile([P, NCS], dt.float32, tag="t_", name=f"t_{mo}_{sc}")
            nc.vector.tensor_tensor(t_[:], ph[:], c1_bc[:, ch], OP.mult)
            s_ = p4.tile([P, NCS], dt.float32, tag="s_", name=f"s_{mo}_{sc}")
            nc.scalar.activation(s_[:], t_[:], AF.Sin,
                                 scale=alpha_sb[:, mo:mo + 1], bias=ab1_sb[:, mo:mo + 1])
            h_ = p4.tile([P, NCS], dt.float32, tag="h_", name=f"h_{mo}_{sc}")
            nc.scalar.activation(h_[:], t_[:], AF.Identity, bias=b1_sb[:, mo:mo + 1])
            sq_ = p4.tile([P, NCS], dt.float32, tag="sq_", name=f"sq_{mo}_{sc}")
            nc.scalar.activation(sq_[:], s_[:], AF.Square)
            nc.vector.scalar_tensor_tensor(h2sb[:, mo, ch], sq_[:],
                                           gam_sb[:, mo:mo + 1], h_[:],
                                           op0=OP.mult, op1=OP.add)
            am_ = p4.tile([P, NCS], dt.float32, tag="am_", name=f"am_{mo}_{sc}")
            nc.vector.scalar_tensor_tensor(am_[:], h2sb[:, mo, ch], -1.0,
                                           h2sb[:, mo, ch], op0=OP.mult, op1=OP.max)
            nc.vector.tensor_tensor(M_acc[:, ch], M_acc[:, ch], am_[:], OP.max)

        def w1_load(mo, key):
            wblk = p4.tile([P, KD, P], dt.bfloat16, tag="wblk", name=f"w1_{key}")
            nc.sync.dma_start(out=wblk[:], in_=w1_d.ap()[mo])
            return wblk

        # first token half for the first 8 mo rows while group B's quant lands
        NSPLIT = 8
        for mo in range(NSPLIT):
            ff1_unit(mo, 0, w1_load(mo, f"a{mo}"))
        for mo in range(NSPLIT):
            ff1_unit(mo, 1, w1_load(mo, f"b{mo}"))
        for mo in range(NSPLIT, KF):
            wblk = w1_load(mo, f"c{mo}")
            for sc in range(NOS):
                ff1_unit(mo, sc, wblk)

        # cross-partition absmax via PE transpose + free-axis reduce
        m_tok = p4m.tile([P, ST], dt.float32)
        for c in range(ST):
            pmt = p4ps.tile([P, P], dt.float32, tag="pmt", bufs=2, name=f"pmt_{c}")
            nc.tensor.transpose(pmt[:], M_acc[:, ts(c, P)], identf[:])
            nc.vector.tensor_reduce(m_tok[:, c:c + 1], pmt[:], axis=mybir.AxisListType.X, op=OP.max)
        mcl2 = p4m.tile([P, ST], dt.float32)
        nc.vector.tensor_scalar(mcl2[:], m_tok[:], 1e-5, None, op0=OP.max)
        rc2 = p4m.tile([P, ST], dt.float32)
        nc.vector.reciprocal(rc2[:], mcl2[:])
        sc2_tok = p4m.tile([P, ST], dt.float32)
        nc.vector.tensor_scalar(sc2_tok[:], rc2[:], 127.0, None, op0=OP.mult)
        nc.vector.tensor_scalar(c2_tok[:], mcl2[:], ws2 / 127.0, None, op0=OP.mult)
        sc2row = p4m.tile([1, S], dt.float32)
        for mt in range(ST):
            eng = nc.scalar if mt % 2 else nc.sync
            eng.dma_start(out=sc2row[0:1, ts(mt, P)], in_=sc2_tok[:, mt:mt + 1])
        for sc in range(NOS):
            pb = p4ps.tile([P, NCS], dt.float32, tag="pb", bufs=2, name=f"pb4_{sc}")
            nc.tensor.matmul(pb[:], ones_1[:], sc2row[:, ts(sc, NCS)], start=True, stop=True)
            nc.vector.tensor_copy(sc2_bc[:, ts(sc, NCS)], pb[:])
        p4ps.release()
        p4m.release()
        p4.release()
        qp.release()  # x2qT, c1_bc dead

        # ===== phase 5: ff2; h2 quantized to ints in place (bf16) =====
        p6 = tc.alloc_tile_pool(name="p6", bufs=3)
        p6q = tc.alloc_tile_pool(name="p6q", bufs=2)
        p6ps = tc.alloc_tile_pool(name="p6ps", bufs=1, space="PSUM")

        for ko in range(KF):
            m1 = p6q.tile([P, S], dt.float32, tag="m1", bufs=2, name=f"m1_{ko}")
            nc.vector.tensor_tensor(m1[:], h2sb[:, ko, :], sc2_bc[:], OP.mult)
            nc.vector.tensor_scalar(h2sb[:, ko, :], m1[:], MAGIC, MAGIC,
                                    op0=OP.add, op1=OP.subtract)

        for no in range(NOD):
            psy = [p6ps.tile([P, NCD], dt.float32, tag=f"y{mt}", name=f"psy6_{no}_{mt}")
                   for mt in range(ST)]
            xchs = []
            for mt in range(ST):  # prefetch residual chunks (bf16)
                xch = p6.tile([P, NCD], dt.bfloat16, tag="xch", bufs=2 * ST,
                              name=f"xch6_{no}_{mt}")
                nc.scalar.dma_start(out=xch[:], in_=xb2_d.ap()[mt][:, ts(no, NCD)])
                xchs.append(xch)
            for ko in range(KF):
                wch = p6.tile([P, NCD], dt.bfloat16, tag="wch", bufs=4)
                nc.sync.dma_start(out=wch[:], in_=w2_d.ap()[no, ko])
                for mt in range(ST):
                    nc.tensor.matmul(psy[mt][:], h2sb[:, ko, ts(mt, P)], wch[:],
                                     start=(ko == 0), stop=(ko == KF - 1))
            for mt in range(ST):
                oe1 = p6.tile([P, NCD], dt.float32, tag="oe1", bufs=2, name=f"oe1_{no}_{mt}")
                nc.scalar.activation(oe1[:], psy[mt][:], AF.Identity,
                                     scale=c2_tok[:, mt:mt + 1])
                oe = p6.tile([P, NCD], dt.float32, tag="oe", bufs=2, name=f"oe_{no}_{mt}")
                nc.vector.tensor_tensor(oe[:], oe1[:], xchs[mt][:], OP.add)
                nc.scalar.dma_start(out=out_d.ap()[ts(mt, P), ts(no, NCD)], in_=oe[:])
        p6ps.release()
        p6q.release()
        p6.release()
        hp.release()
        cp.release()
    return nc


# ---------------------------------------------------------------- driver

def _get_compiled(key, S, D, H, DFF, ws1, ws2):
    if key in _CACHE:
        return _CACHE[key]
    from concourse import bacc

    nc = bacc.Bacc("TRN2", target_bir_lowering=False, debug=False, num_devices=NCORES)
    build_program(nc, S=S, D=D, H=H, DFF=DFF, ws1=ws1, ws2=ws2)
    nc.compile()
    _CACHE[key] = nc
    return nc


def make_in_maps(inputs):
    src = np.asarray(inputs["src"], dtype=np.float32)
    B, S, D = src.shape
    H = H_FULL
    DFF = inputs["ff1_w"].shape[0]
    arrays, ws1, ws2 = _prep_arrays(inputs, S, D, H, DFF)
    srcob = src + np.asarray(inputs["out_proj_b"], dtype=np.float32)[None, None, :]
    in_maps = []
    for c in range(NCORES):
        m = dict(arrays)
        m["src"] = np.ascontiguousarray(src[c])
        m["srcob"] = np.ascontiguousarray(srcob[c])
        in_maps.append(m)
    return in_maps, (S, D, H, DFF, ws1, ws2)


def kernel(**inputs):
    from concourse.bass_utils import run_bass_kernel_spmd

    in_maps, (S, D, H, DFF, ws1, ws2) = make_in_maps(inputs)
    assert np.asarray(inputs["src"]).shape[0] == NCORES
    nc = _get_compiled(("full", S, D, H, DFF, ws1, ws2), S, D, H, DFF, ws1, ws2)
    res = run_bass_kernel_spmd(nc, in_maps, core_ids=list(range(NCORES)))
    out = np.stack([res.results[c]["out"] for c in range(NCORES)], axis=0)
    return out.astype(np.float32)


# revision 20
# speedup vs baseline: 1.1524x; 1.1524x over previous
"""BitTransformerEncoderLayer on 8 TRN2 NeuronCores.

Strategy: pure data parallelism over batch (B=8 == n_cores). Each core runs the
full layer for one batch element; no collectives. BitLinear matmuls run as exact
integer arithmetic in bf16 (ternary weights quantized on host; activations
rounded to ints <=127 on device via the fp32 magic-constant trick), with the
fp32 scales folded into PSUM eviction.

v3:
- all 128x128 transposes (x2T, vT, x2qT) on DMA xbar (dma_start_transpose)
- softmax denominator fully off the PE: DVE tt-sum + GpSimd partition_all_reduce
- qkv PSUM evictions on DVE (ACT keeps only exp); deeper PSUM buffering
- norm1_w folded into in_proj weights on host
- phase 3: out_proj in two 4-token-tile groups; group A's rmsnorm2/quant chains
  interleave with group B's matmul blocks; group B's chains hide under ff1's
  first 8 rows (whose h2 spills to DRAM and is read back for ff2 - the rest of
  h2 stays in SBUF as bf16, quantized in place). Per-token scale broadcasts via
  tiny PE transposes + 1-row matmuls (no SBUF row-gather DMAs).
- DMA queues: weights on sync, xbar transposes on scalar, bulk loads/spills on
  gpsimd (SWDGE)

kernel(**inputs) takes the FULL unsharded inputs and returns the FULL output.
"""

import numpy as np

P = 128
EPS = 1e-8
MAGIC = 12582912.0  # 1.5 * 2**23: fp32 add/sub rounds to nearest-even integer
NCORES = 8
NSPLIT = 8            # ff1 rows whose h2 spills to DRAM (overlap window)

B_FULL, S_FULL, D_FULL, H_FULL, DFF_FULL = 8, 1024, 2048, 16, 8192

_CACHE = {}


# ---------------------------------------------------------------- host prep

def _quant_w(w):
    scale = np.maximum(np.mean(np.abs(w), dtype=np.float32), np.float32(1e-5))
    q = np.clip(np.round(w / scale), -1.0, 1.0).astype(np.float32)
    return q, float(scale)


def _lhsT_blocks(w):
    """w [M, K] -> [M/P, P(k), K/P, P(m)]; [mo, :, ko, :] = w-block(mo, ko).T"""
    M, K = w.shape
    t = w.reshape(M // P, P, K // P, P)  # [mo, pm, ko, pk]
    return np.ascontiguousarray(t.transpose(0, 3, 2, 1))


def _rhs_chunks(w, nch):
    """w [N, K] -> [N/nch, K/P, P, nch]; [no, ko, p, j] = w[no*nch+j, ko*P+p]"""
    N, K = w.shape
    t = w.reshape(N // nch, nch, K // P, P)  # [no, j, ko, p]
    return np.ascontiguousarray(t.transpose(0, 2, 3, 1))


def _per_part(v):
    """[M] -> [P, M/P]; out[p, mo] = v[mo*P + p]"""
    return np.ascontiguousarray(v.reshape(-1, P).T)


def _bcast_row(v):
    return np.ascontiguousarray(np.broadcast_to(v[None, :], (P, v.shape[0])))


def _prep_arrays(inputs, S, D, H, DFF):
    import ml_dtypes

    bf16 = ml_dtypes.bfloat16
    f32 = np.float32
    g = lambda k: np.asarray(inputs[k], dtype=f32)

    w1q, ws1 = _quant_w(g("ff1_w"))   # [DFF, D]
    w2q, ws2 = _quant_w(g("ff2_w"))   # [D, DFF]
    ncd = min(512, D)

    w_in_eff = g("in_proj_w") * g("norm1_w")[None, :]  # fold rmsnorm1 weight

    arrays = {
        "w_in_blk": _lhsT_blocks(w_in_eff).astype(bf16),        # [3D/P, P, D/P, P]
        "wo_chunk": _rhs_chunks(g("out_proj_w"), ncd).astype(bf16),
        "w1_blk": _lhsT_blocks(w1q).astype(bf16),                # [DFF/P, P, D/P, P]
        "w2_chunk": _rhs_chunks(w2q, ncd).astype(bf16),          # [D/ncd, DFF/P, P, ncd]
        "bias_in": _per_part(g("in_proj_b")).astype(f32),        # [P, 3D/P]
        "b1_t": _per_part(g("ff1_b")).astype(f32),               # [P, DFF/P]
        "alpha_t": _per_part(g("alpha")).astype(f32),
        "ab1_t": _per_part((g("alpha") * g("ff1_b")).astype(f32)),
        "gamma_t": _per_part((1.0 / (g("beta") + np.float32(1e-9))).astype(f32)),
        "n2w_bc": _bcast_row(g("norm2_w")).astype(f32),          # [P, D]
        "b2_bc": _bcast_row(g("ff2_b")).astype(f32),
    }
    return arrays, ws1, ws2


# ---------------------------------------------------------------- device program

def build_program(nc, *, S, D, H, DFF, ws1, ws2):
    import concourse.mybir as mybir
    import concourse.tile as tile
    from concourse.bass import ts
    from concourse.bass_isa import ReduceOp
    from concourse.masks import make_identity

    dt = mybir.dt
    AF = mybir.ActivationFunctionType
    OP = mybir.AluOpType

    DH = D // H
    assert DH == P, "layout assumes head dim == 128"
    ST = S // P           # token tiles
    KD = D // P           # D contraction tiles
    KF = DFF // P         # DFF contraction tiles / ff1 out tiles
    NCD = min(512, D)     # fo chunk for out_proj/ff2 (psum-bank sized)
    NOD = D // NCD
    NCS = min(512, S)     # s chunk
    NOS = S // NCS
    MTG = ST // 2         # token tiles per out_proj group
    inv_sqrt_dh = float(1.0 / np.sqrt(DH))

    # ---- DRAM I/O ----
    src_d = nc.dram_tensor("src", [S, D], dt.float32, kind="ExternalInput")
    srcob_d = nc.dram_tensor("srcob", [S, D], dt.float32, kind="ExternalInput")
    w_in_d = nc.dram_tensor("w_in_blk", [3 * KD, P, KD, P], dt.bfloat16, kind="ExternalInput")
    wo_d = nc.dram_tensor("wo_chunk", [NOD, KD, P, NCD], dt.bfloat16, kind="ExternalInput")
    w1_d = nc.dram_tensor("w1_blk", [KF, P, KD, P], dt.bfloat16, kind="ExternalInput")
    w2_d = nc.dram_tensor("w2_chunk", [NOD, KF, P, NCD], dt.bfloat16, kind="ExternalInput")
    bin_d = nc.dram_tensor("bias_in", [P, 3 * KD], dt.float32, kind="ExternalInput")
    b1_d = nc.dram_tensor("b1_t", [P, KF], dt.float32, kind="ExternalInput")
    alpha_d = nc.dram_tensor("alpha_t", [P, KF], dt.float32, kind="ExternalInput")
    ab1_d = nc.dram_tensor("ab1_t", [P, KF], dt.float32, kind="ExternalInput")
    gam_d = nc.dram_tensor("gamma_t", [P, KF], dt.float32, kind="ExternalInput")
    n2w_d = nc.dram_tensor("n2w_bc", [P, D], dt.float32, kind="ExternalInput")
    b2_d = nc.dram_tensor("b2_bc", [P, D], dt.float32, kind="ExternalInput")
    out_d = nc.dram_tensor("out", [S, D], dt.float32, kind="ExternalOutput")
    xb2_d = nc.dram_tensor("xb2_spill", [ST, P, D], dt.bfloat16)
    h2a_d = nc.dram_tensor("h2a_spill", [NSPLIT, P, S], dt.bfloat16)

    with tile.TileContext(nc) as tc:
        # ---------- persistent constants ----------
        cp = tc.alloc_tile_pool(name="consts", bufs=1)
        identf = cp.tile([P, P], dt.float32)
        make_identity(nc, identf)
        ones_1b = cp.tile([1, P], dt.bfloat16)
        nc.any.memset(ones_1b[:], 1.0)
        magic_col = cp.tile([P, 1], dt.float32)
        nc.any.memset(magic_col[:], MAGIC)
        bin_sb = cp.tile([P, 3 * KD], dt.float32)
        nc.gpsimd.dma_start(out=bin_sb[:], in_=bin_d.ap())
        b1_sb = cp.tile([P, KF], dt.float32)
        nc.gpsimd.dma_start(out=b1_sb[:], in_=b1_d.ap())
        alpha_sb = cp.tile([P, KF], dt.float32)
        nc.gpsimd.dma_start(out=alpha_sb[:], in_=alpha_d.ap())
        ab1_sb = cp.tile([P, KF], dt.float32)
        nc.gpsimd.dma_start(out=ab1_sb[:], in_=ab1_d.ap())
        gam_sb = cp.tile([P, KF], dt.float32)
        nc.gpsimd.dma_start(out=gam_sb[:], in_=gam_d.ap())
        c2_tok = cp.tile([P, ST], dt.float32)   # filled in phase 4
        sc2_bc = cp.tile([P, S], dt.float32)    # filled in phase 4

        # ================= phase 1: rmsnorm1 (n1w folded) + DMA transpose =====
        xp = tc.alloc_tile_pool(name="x2T_pool", bufs=1)
        x2T = xp.tile([P, KD, S], dt.bfloat16)

        p1 = tc.alloc_tile_pool(name="p1", bufs=2)
        for mt in range(ST):
            xt = p1.tile([P, D], dt.float32, tag="xt", bufs=3)
            nc.gpsimd.dma_start(out=xt[:], in_=src_d.ap()[ts(mt, P), :])
            sq = p1.tile([P, D], dt.float32, tag="sq", bufs=2)
            ss = p1.tile([P, 1], dt.float32, tag="ss")
            nc.scalar.activation(sq[:], xt[:], AF.Square, accum_out=ss[:])
            ms = p1.tile([P, 1], dt.float32, tag="ms")
            nc.vector.tensor_scalar(ms[:], ss[:], 1.0 / D, EPS, op0=OP.mult, op1=OP.add)
            rt = p1.tile([P, 1], dt.float32, tag="rt")
            nc.scalar.activation(rt[:], ms[:], AF.Sqrt)
            rs = p1.tile([P, 1], dt.float32, tag="rs")
            nc.vector.reciprocal(rs[:], rt[:])
            x2 = p1.tile([P, D], dt.bfloat16, tag="x2", bufs=3)
            nc.scalar.activation(x2[:], xt[:], AF.Copy, scale=rs[:])
            nc.scalar.dma_start_transpose(out=x2T[:, :, ts(mt, P)], in_=x2[:])
        p1.release()

        # ================= phase 2: fused in_proj + attention =================
        op_ = tc.alloc_tile_pool(name="oT_pool", bufs=1, side="right")
        oT_all = op_.tile([P, KD, S], dt.bfloat16)

        p2w = tc.alloc_tile_pool(name="p2w", bufs=2)
        p2 = tc.alloc_tile_pool(name="p2", bufs=2)
        p2a = tc.alloc_tile_pool(name="p2a", bufs=2, space="PSUM")
        p2b = tc.alloc_tile_pool(name="p2b", bufs=3, space="PSUM")

        def attn_tail(h, es, vT, expT):
            # denominator: partition-sum of es (already tt-summed) on GpSimd,
            # reciprocal on DVE; folded into the o^T eviction. One head late.
            for sc in range(NOS):
                esb = p2.tile([P, NCS], dt.float32, tag="esb", name=f"esb_{h}_{sc}")
                nc.gpsimd.partition_all_reduce(esb[:], es[sc][:], P, ReduceOp.add)
                rb = p2.tile([P, NCS], dt.float32, tag="rb", name=f"rb_{h}_{sc}")
                nc.vector.reciprocal(rb[:], esb[:])
                po = p2a.tile([P, NCS], dt.float32, tag="po", bufs=2, name=f"po_{h}_{sc}")
                for tt in range(ST):
                    nc.tensor.matmul(po[:], vT[:, tt, :], expT[:, tt, ts(sc, NCS)],
                                     start=(tt == 0), stop=(tt == ST - 1))
                nc.vector.tensor_tensor(oT_all[:, h, ts(sc, NCS)], po[:], rb[:], OP.mult)

        prev = None
        for h in range(H):
            qkv = []
            for j, mo in ((0, h), (1, KD + h), (2, 2 * KD + h)):
                wblk = p2w.tile([P, KD, P], dt.bfloat16, tag="wblk", bufs=3)
                nc.sync.dma_start(out=wblk[:], in_=w_in_d.ap()[mo])
                dest = p2.tile([P, S], dt.bfloat16, tag=f"qkv{j}", name=f"qkv{j}_{h}")
                for sc in range(NOS):
                    ps = p2a.tile([P, NCS], dt.float32, tag="mmps", bufs=3,
                                  name=f"qkvps_{h}_{j}_{sc}")
                    for ko in range(KD):
                        nc.tensor.matmul(ps[:], wblk[:, ko, :], x2T[:, ko, ts(sc, NCS)],
                                         start=(ko == 0), stop=(ko == KD - 1))
                    nc.vector.tensor_scalar(dest[:, ts(sc, NCS)], ps[:],
                                            bin_sb[:, mo:mo + 1], None, op0=OP.add)
                qkv.append(dest)
            q, k, v = qkv
            # scores^T -> exp; accumulate the tt-sum for the denominator on DVE
            expT = p2.tile([P, ST, S], dt.bfloat16, tag="expT", name=f"expT_{h}")
            es = [p2.tile([P, NCS], dt.float32, tag=f"es{sc}", name=f"es_{h}_{sc}")
                  for sc in range(NOS)]
            for tt in range(ST):
                for sc in range(NOS):
                    ps = p2b.tile([P, NCS], dt.float32, tag="scps", bufs=3,
                                  name=f"scps_{h}_{tt}_{sc}")
                    nc.tensor.matmul(ps[:], k[:, ts(tt, P)], q[:, ts(sc, NCS)],
                                     start=True, stop=True)
                    nc.scalar.activation(expT[:, tt, ts(sc, NCS)], ps[:], AF.Exp,
                                         scale=inv_sqrt_dh)
                    if tt == 0:
                        nc.vector.tensor_copy(es[sc][:], expT[:, 0, ts(sc, NCS)])
                    else:
                        nc.vector.tensor_tensor(es[sc][:], es[sc][:],
                                                expT[:, tt, ts(sc, NCS)], OP.add)
            # v^T via DMA xbar transpose
            vT = p2.tile([P, ST, P], dt.bfloat16, tag="vT", name=f"vT_{h}")
            nc.scalar.dma_start_transpose(out=vT[:, :, :], in_=v[:])
            if prev is not None:
                attn_tail(*prev)
            prev = (h, es, vT, expT)
        attn_tail(*prev)
        p2b.release()
        p2a.release()
        p2.release()
        p2w.release()
        xp.release()  # x2T dead

        # ===== phase 3: out_proj + residual + rmsnorm2 + quant, 2 mt-groups =====
        qp = tc.alloc_tile_pool(name="x2qT_pool", bufs=1)
        x2qT = qp.tile([P, KD, S], dt.bfloat16)
        c1_bc = qp.tile([P, S], dt.float32)

        p3c = tc.alloc_tile_pool(name="p3c", bufs=1)
        p3x = tc.alloc_tile_pool(name="p3x", bufs=1)
        p3 = tc.alloc_tile_pool(name="p3", bufs=3)
        p3q = tc.alloc_tile_pool(name="p3q", bufs=2)
        p3ps = tc.alloc_tile_pool(name="p3ps", bufs=1, space="PSUM")
        n2w_sb = p3c.tile([P, D], dt.float32)
        nc.gpsimd.dma_start(out=n2w_sb[:], in_=n2w_d.ap())
        b2_sb = p3c.tile([P, D], dt.float32)
        nc.gpsimd.dma_start(out=b2_sb[:], in_=b2_d.ap())

        xg, ssl, c1c_of = {}, {}, {}

        def quant_chain(mt):
            xt = xg[mt]
            ss = p3q.tile([P, 1], dt.float32, tag="ss", bufs=4, name=f"ssq_{mt}")
            nc.vector.tensor_reduce(ss[:], ssl[mt][:], axis=mybir.AxisListType.X, op=OP.add)
            ms = p3q.tile([P, 1], dt.float32, tag="ms", bufs=4)
            nc.vector.tensor_scalar(ms[:], ss[:], 1.0 / D, EPS, op0=OP.mult, op1=OP.add)
            rt = p3q.tile([P, 1], dt.float32, tag="rt", bufs=4)
            nc.scalar.activation(rt[:], ms[:], AF.Sqrt)
            rs = p3q.tile([P, 1], dt.float32, tag="rs", bufs=4)
            nc.vector.reciprocal(rs[:], rt[:])
            x2 = p3q.tile([P, D], dt.float32, tag="x2", bufs=2)
            nc.vector.scalar_tensor_tensor(x2[:], xt[:], rs[:], n2w_sb[:],
                                           op0=OP.mult, op1=OP.mult)
            mx = p3q.tile([P, 1], dt.float32, tag="mx", bufs=4)
            nc.vector.tensor_reduce(mx[:], x2[:], axis=mybir.AxisListType.X,
                                    op=OP.max, apply_absolute_value=True)
            mcl = p3q.tile([P, 1], dt.float32, tag="mcl", bufs=4)
            nc.vector.tensor_scalar(mcl[:], mx[:], 1e-5, None, op0=OP.max)
            rc = p3q.tile([P, 1], dt.float32, tag="rc", bufs=4)
            nc.vector.reciprocal(rc[:], mcl[:])
            sc1 = p3q.tile([P, 1], dt.float32, tag="sc1", bufs=4)
            nc.vector.tensor_scalar(sc1[:], rc[:], 127.0, None, op0=OP.mult)
            c1c = p3q.tile([P, 1], dt.float32, tag="c1c", bufs=ST, name=f"c1c_{mt}")
            nc.vector.tensor_scalar(c1c[:], mcl[:], ws1 / 127.0, None, op0=OP.mult)
            c1c_of[mt] = c1c
            t1 = p3q.tile([P, D], dt.float32, tag="t1", bufs=2)
            nc.scalar.activation(t1[:], x2[:], AF.Identity, scale=sc1[:], bias=magic_col[:])
            xq = p3q.tile([P, D], dt.bfloat16, tag="xq", bufs=2)
            nc.vector.tensor_scalar(xq[:], t1[:], MAGIC, None, op0=OP.subtract)
            nc.scalar.dma_start_transpose(out=x2qT[:, :, ts(mt, P)], in_=xq[:])

        def bcast_col(col, dest_slice, trtag, bctag, pool, key):
            """col [P,1] f32 (partition=token) -> dest [P,128] f32 broadcast."""
            pt = pool.tile([1, P], dt.float32, tag=trtag, bufs=2, name=f"bt_{key}")
            nc.tensor.transpose(pt[:], col[:], identf[:])
            row = p3q.tile([1, P], dt.bfloat16, tag="brow", bufs=4, name=f"br_{key}")
            nc.scalar.activation(row[:], pt[:], AF.Copy)
            pbc = pool.tile([P, P], dt.float32, tag=bctag, bufs=2, name=f"bp_{key}")
            nc.tensor.matmul(pbc[:], ones_1b[:], row[:], start=True, stop=True)
            nc.vector.tensor_copy(dest_slice, pbc[:])

        def no_block(g, no, mts, psy):
            for ko in range(KD):
                wch = p3.tile([P, NCD], dt.bfloat16, tag="wch", bufs=6)
                nc.sync.dma_start(out=wch[:], in_=wo_d.ap()[no, ko])
                for mt in mts:
                    nc.tensor.matmul(psy[mt][:], oT_all[:, ko, ts(mt, P)], wch[:],
                                     start=(ko == 0), stop=(ko == KD - 1))
            ch = ts(no, NCD)
            for mt in mts:
                nc.vector.tensor_tensor(xg[mt][:, ch], xg[mt][:, ch], psy[mt][:], OP.add)
                if no < NOD - 1:
                    psy[mt] = p3ps.tile([P, NCD], dt.float32, tag=f"y{mt % MTG}",
                                        bufs=2, name=f"psy3_{g}_{mt}_{no + 1}")
            for mt in mts:
                xo = p3.tile([P, NCD], dt.bfloat16, tag="xo", bufs=4,
                             name=f"xo_{g}_{no}_{mt}")
                nc.vector.tensor_tensor(xo[:], xg[mt][:, ch], b2_sb[:, ch], OP.add)
                nc.gpsimd.dma_start(out=xb2_d.ap()[mt][:, ch], in_=xo[:])
                sqt = p3.tile([P, NCD], dt.float32, tag="sqt", bufs=4,
                              name=f"sqt_{g}_{no}_{mt}")
                nc.scalar.activation(sqt[:], xg[mt][:, ch], AF.Square,
                                     accum_out=ssl[mt][:, no:no + 1])

        def group_setup(g):
            mts = list(range(g * MTG, (g + 1) * MTG))
            for mt in mts:
                xg[mt] = p3x.tile([P, D], dt.float32, tag=f"xg{mt % MTG}", bufs=1,
                                  name=f"xg_{mt}")
                nc.gpsimd.dma_start(out=xg[mt][:], in_=srcob_d.ap()[ts(mt, P), :])
                ssl[mt] = p3q.tile([P, NOD], dt.float32, tag=f"ssl{mt % MTG}", bufs=2,
                                   name=f"ssl_{mt}")
            psy = {mt: p3ps.tile([P, NCD], dt.float32, tag=f"y{mt % MTG}", bufs=2,
                                 name=f"psy3_{g}_{mt}")
                   for mt in mts}
            return mts, psy

        mts0, psy0 = group_setup(0)
        for no in range(NOD):
            no_block(0, no, mts0, psy0)
        mts1, psy1 = group_setup(1)
        for no in range(NOD):
            no_block(1, no, mts1, psy1)
            quant_chain(mts0[no])       # group-0 chains hide under group-1 MMs
        for mt in mts0:                  # c1 broadcast, token half 0
            bcast_col(c1c_of[mt], c1_bc[:, ts(mt, P)], "y2", "y3", p3ps, f"c1_{mt}")
        p3ps.release()
        op_.release()  # oT_all dead

        # ======= phase 4a: ff1 rows 0..NSPLIT-1 (h2 -> DRAM); group-1 chains ===
        p4 = tc.alloc_tile_pool(name="p4", bufs=2, side="right")
        p4m = tc.alloc_tile_pool(name="p4m", bufs=1, side="right")
        p4ps = tc.alloc_tile_pool(name="p4ps", bufs=3, space="PSUM")
        M_acc = p4m.tile([P, S], dt.float32)
        nc.any.memset(M_acc[:], 0.0)

        def ff1_unit(mo, sc, wblk, h2dest, h2spill):
            ph = p4ps.tile([P, NCS], dt.float32, tag="ph", name=f"ph_{mo}_{sc}")
            for ko in range(KD):
                nc.tensor.matmul(ph[:], wblk[:, ko, :], x2qT[:, ko, ts(sc, NCS)],
                                 start=(ko == 0), stop=(ko == KD - 1))
            ch = ts(sc, NCS)
            t_ = p4.tile([P, NCS], dt.float32, tag="t_", name=f"t_{mo}_{sc}")
            nc.vector.tensor_tensor(t_[:], ph[:], c1_bc[:, ch], OP.mult)
            s_ = p4.tile([P, NCS], dt.float32, tag="s_", name=f"s_{mo}_{sc}")
            nc.scalar.activation(s_[:], t_[:], AF.Sin,
                                 scale=alpha_sb[:, mo:mo + 1], bias=ab1_sb[:, mo:mo + 1])
            h_ = p4.tile([P, NCS], dt.float32, tag="h_", name=f"h_{mo}_{sc}")
            nc.scalar.activation(h_[:], t_[:], AF.Identity, bias=b1_sb[:, mo:mo + 1])
            sq_ = p4.tile([P, NCS], dt.float32, tag="sq_", name=f"sq_{mo}_{sc}")
            nc.scalar.activation(sq_[:], s_[:], AF.Square)
            nc.vector.scalar_tensor_tensor(h2dest, sq_[:], gam_sb[:, mo:mo + 1], h_[:],
                                           op0=OP.mult, op1=OP.add)
            if h2spill is not None:
                nc.gpsimd.dma_start(out=h2spill, in_=h2dest)
            am_ = p4.tile([P, NCS], dt.float32, tag="am_", name=f"am_{mo}_{sc}")
            nc.vector.scalar_tensor_tensor(am_[:], h2dest, -1.0, h2dest,
                                           op0=OP.mult, op1=OP.max)
            nc.vector.tensor_tensor(M_acc[:, ch], M_acc[:, ch], am_[:], OP.max)

        def w1_load(mo, key):
            wblk = p4.tile([P, KD, P], dt.bfloat16, tag="wblk", bufs=2, name=f"w1_{key}")
            nc.sync.dma_start(out=wblk[:], in_=w1_d.ap()[mo])
            return wblk

        def ff1_spill_unit(mo, sc, wblk):
            h2t = p4.tile([P, NCS], dt.bfloat16, tag="h2t", bufs=3,
                          name=f"h2t_{mo}_{sc}")
            ff1_unit(mo, sc, wblk, h2t[:], h2a_d.ap()[mo][:, ts(sc, NCS)])

        for mo in range(NSPLIT):   # token half 0; group-1 chains interleaved
            ff1_spill_unit(mo, 0, w1_load(mo, f"a{mo}"))
            if mo < MTG:
                quant_chain(mts1[mo])
        for mt in mts1:            # c1 broadcast, token half 1
            bcast_col(c1c_of[mt], c1_bc[:, ts(mt, P)], "pmt", "pb4", p4ps, f"c1_{mt}")
        for mo in range(NSPLIT):   # token half 1
            ff1_spill_unit(mo, 1, w1_load(mo, f"b{mo}"))
        p3q.release()
        p3.release()
        p3x.release()
        p3c.release()

        # ======= phase 4b: ff1 rows NSPLIT.. with h2 resident in SBUF =========
        hp = tc.alloc_tile_pool(name="h2_pool", bufs=1)
        h2sb = hp.tile([P, KF, S], dt.bfloat16)

        for mo in range(NSPLIT, KF):
            wblk = w1_load(mo, f"c{mo}")
            for sc in range(NOS):
                ff1_unit(mo, sc, wblk, h2sb[:, mo, ts(sc, NCS)], None)

        # cross-partition absmax via PE transpose + free-axis reduce
        m_tok = p4m.tile([P, ST], dt.float32)
        for c in range(ST):
            pmt = p4ps.tile([P, P], dt.float32, tag="pmt", bufs=2, name=f"pmt_{c}")
            nc.tensor.transpose(pmt[:], M_acc[:, ts(c, P)], identf[:])
            nc.vector.tensor_reduce(m_tok[:, c:c + 1], pmt[:], axis=mybir.AxisListType.X, op=OP.max)
        mcl2 = p4m.tile([P, ST], dt.float32)
        nc.vector.tensor_scalar(mcl2[:], m_tok[:], 1e-5, None, op0=OP.max)
        rc2 = p4m.tile([P, ST], dt.float32)
        nc.vector.reciprocal(rc2[:], mcl2[:])
        sc2_tok = p4m.tile([P, ST], dt.float32)
        nc.vector.tensor_scalar(sc2_tok[:], rc2[:], 127.0, None, op0=OP.mult)
        nc.vector.tensor_scalar(c2_tok[:], mcl2[:], ws2 / 127.0, None, op0=OP.mult)
        brow4 = tc.alloc_tile_pool(name="brow4", bufs=1, side="right")

        def bcast_col4(col, dest_slice, key):
            pt = p4ps.tile([1, P], dt.float32, tag="pmt", bufs=2, name=f"bt4_{key}")
            nc.tensor.transpose(pt[:], col[:], identf[:])
            row = brow4.tile([1, P], dt.bfloat16, tag="brow", bufs=4, name=f"br4_{key}")
            nc.scalar.activation(row[:], pt[:], AF.Copy)
            pbc = p4ps.tile([P, P], dt.float32, tag="pb4", bufs=2, name=f"bp4_{key}")
            nc.tensor.matmul(pbc[:], ones_1b[:], row[:], start=True, stop=True)
            nc.vector.tensor_copy(dest_slice, pbc[:])

        for mt in range(ST):  # per-token ff2 quant scale broadcast
            bcast_col4(sc2_tok[:, mt:mt + 1], sc2_bc[:, ts(mt, P)], f"s2_{mt}")
        brow4.release()
        p4ps.release()
        p4m.release()
        p4.release()

        # ===== phase 5: ff2; h2 quantized to ints in place (bf16) =====
        p6 = tc.alloc_tile_pool(name="p6", bufs=3, side="right")
        p6ps = tc.alloc_tile_pool(name="p6ps", bufs=1, space="PSUM")

        for ko in range(NSPLIT):  # read the spilled rows back into h2sb
            nc.gpsimd.dma_start(out=h2sb[:, ko, :], in_=h2a_d.ap()[ko])
        for ko in range(KF):
            src_ko = h2sb[:, ko, :]
            for sch in range(NOS):
                m1 = p6.tile([P, NCS], dt.float32, tag="m1", bufs=3,
                             name=f"m1_{ko}_{sch}")
                nc.vector.tensor_tensor(m1[:], src_ko[:, ts(sch, NCS)],
                                        sc2_bc[:, ts(sch, NCS)], OP.mult)
                nc.vector.tensor_scalar(src_ko[:, ts(sch, NCS)], m1[:], MAGIC, MAGIC,
                                        op0=OP.add, op1=OP.subtract)

        for no in range(NOD):
            psy = [p6ps.tile([P, NCD], dt.float32, tag=f"y{mt}", name=f"psy6_{no}_{mt}")
                   for mt in range(ST)]
            xchs = []
            for mt in range(ST):  # prefetch residual chunks (bf16)
                xch = p6.tile([P, NCD], dt.bfloat16, tag="xch", bufs=10,
                              name=f"xch6_{no}_{mt}")
                nc.gpsimd.dma_start(out=xch[:], in_=xb2_d.ap()[mt][:, ts(no, NCD)])
                xchs.append(xch)
            for ko in range(KF):
                wch = p6.tile([P, NCD], dt.bfloat16, tag="wch", bufs=12)
                nc.sync.dma_start(out=wch[:], in_=w2_d.ap()[no, ko])
                for mt in range(ST):
                    nc.tensor.matmul(psy[mt][:], h2sb[:, ko, ts(mt, P)], wch[:],
                                     start=(ko == 0), stop=(ko == KF - 1))
            for mt in range(ST):
                oe1 = p6.tile([P, NCD], dt.float32, tag="oe1", bufs=2, name=f"oe1_{no}_{mt}")
                nc.scalar.activation(oe1[:], psy[mt][:], AF.Identity,
                                     scale=c2_tok[:, mt:mt + 1])
                oe = p6.tile([P, NCD], dt.float32, tag="oe", bufs=2, name=f"oe_{no}_{mt}")
                nc.vector.tensor_tensor(oe[:], oe1[:], xchs[mt][:], OP.add)
                nc.gpsimd.dma_start(out=out_d.ap()[ts(mt, P), ts(no, NCD)], in_=oe[:])
        p6ps.release()
        p6.release()
        hp.release()
        qp.release()
        cp.release()
    return nc


# ---------------------------------------------------------------- driver

def _get_compiled(key, S, D, H, DFF, ws1, ws2):
    if key in _CACHE:
        return _CACHE[key]
    from concourse import bacc

    nc = bacc.Bacc("TRN2", target_bir_lowering=False, debug=False, num_devices=NCORES)
    build_program(nc, S=S, D=D, H=H, DFF=DFF, ws1=ws1, ws2=ws2)
    nc.compile()
    _CACHE[key] = nc
    return nc


def make_in_maps(inputs):
    src = np.asarray(inputs["src"], dtype=np.float32)
    B, S, D = src.shape
    H = H_FULL
    DFF = inputs["ff1_w"].shape[0]
    arrays, ws1, ws2 = _prep_arrays(inputs, S, D, H, DFF)
    srcob = src + np.asarray(inputs["out_proj_b"], dtype=np.float32)[None, None, :]
    in_maps = []
    for c in range(NCORES):
        m = dict(arrays)
        m["src"] = np.ascontiguousarray(src[c])
        m["srcob"] = np.ascontiguousarray(srcob[c])
        in_maps.append(m)
    return in_maps, (S, D, H, DFF, ws1, ws2)


def kernel(**inputs):
    from concourse.bass_utils import run_bass_kernel_spmd

    in_maps, (S, D, H, DFF, ws1, ws2) = make_in_maps(inputs)
    assert np.asarray(inputs["src"]).shape[0] == NCORES
    nc = _get_compiled(("full", S, D, H, DFF, ws1, ws2), S, D, H, DFF, ws1, ws2)
    res = run_bass_kernel_spmd(nc, in_maps, core_ids=list(range(NCORES)))
    out = np.stack([res.results[c]["out"] for c in range(NCORES)], axis=0)
    return out.astype(np.float32)


# revision 22
# speedup vs baseline: 1.1881x; 1.0310x over previous
"""BitTransformerEncoderLayer on 8 TRN2 NeuronCores.

Strategy: pure data parallelism over batch (B=8 == n_cores). Each core runs the
full layer for one batch element; no collectives. BitLinear matmuls run as exact
integer arithmetic in bf16 (ternary weights quantized on host; activations
rounded to ints <=127 on device via the fp32 magic-constant trick), with the
fp32 scales folded into PSUM eviction.

v3:
- all 128x128 transposes (x2T, vT, x2qT) on DMA xbar (dma_start_transpose)
- softmax denominator fully off the PE: DVE tt-sum + GpSimd partition_all_reduce
- qkv PSUM evictions on DVE (ACT keeps only exp); deeper PSUM buffering
- norm1_w folded into in_proj weights on host
- phase 3: out_proj in two 4-token-tile groups; group A's rmsnorm2/quant chains
  interleave with group B's matmul blocks; group B's chains hide under ff1's
  first 8 rows (whose h2 spills to DRAM and is read back for ff2 - the rest of
  h2 stays in SBUF as bf16, quantized in place). Per-token scale broadcasts via
  tiny PE transposes + 1-row matmuls (no SBUF row-gather DMAs).
- DMA queues: weights on sync, xbar transposes on scalar, bulk loads/spills on
  gpsimd (SWDGE)

kernel(**inputs) takes the FULL unsharded inputs and returns the FULL output.
"""

import numpy as np

P = 128
EPS = 1e-8
MAGIC = 12582912.0  # 1.5 * 2**23: fp32 add/sub rounds to nearest-even integer
NCORES = 8
NSPLIT = 8            # ff1 rows whose h2 spills to DRAM (overlap window)

B_FULL, S_FULL, D_FULL, H_FULL, DFF_FULL = 8, 1024, 2048, 16, 8192

_CACHE = {}


# ---------------------------------------------------------------- host prep

def _quant_w(w):
    scale = np.maximum(np.mean(np.abs(w), dtype=np.float32), np.float32(1e-5))
    q = np.clip(np.round(w / scale), -1.0, 1.0).astype(np.float32)
    return q, float(scale)


def _lhsT_blocks(w):
    """w [M, K] -> [M/P, P(k), K/P, P(m)]; [mo, :, ko, :] = w-block(mo, ko).T"""
    M, K = w.shape
    t = w.reshape(M // P, P, K // P, P)  # [mo, pm, ko, pk]
    return np.ascontiguousarray(t.transpose(0, 3, 2, 1))


def _rhs_chunks(w, nch):
    """w [N, K] -> [N/nch, K/P, P, nch]; [no, ko, p, j] = w[no*nch+j, ko*P+p]"""
    N, K = w.shape
    t = w.reshape(N // nch, nch, K // P, P)  # [no, j, ko, p]
    return np.ascontiguousarray(t.transpose(0, 2, 3, 1))


def _per_part(v):
    """[M] -> [P, M/P]; out[p, mo] = v[mo*P + p]"""
    return np.ascontiguousarray(v.reshape(-1, P).T)


def _bcast_row(v):
    return np.ascontiguousarray(np.broadcast_to(v[None, :], (P, v.shape[0])))


def _prep_arrays(inputs, S, D, H, DFF):
    import ml_dtypes

    bf16 = ml_dtypes.bfloat16
    f32 = np.float32
    g = lambda k: np.asarray(inputs[k], dtype=f32)

    w1q, ws1 = _quant_w(g("ff1_w"))   # [DFF, D]
    w2q, ws2 = _quant_w(g("ff2_w"))   # [D, DFF]
    ncd = min(512, D)

    w_in_eff = g("in_proj_w") * g("norm1_w")[None, :]  # fold rmsnorm1 weight

    arrays = {
        "w_in_blk": _lhsT_blocks(w_in_eff).astype(bf16),        # [3D/P, P, D/P, P]
        "wo_chunk": _rhs_chunks(g("out_proj_w"), ncd).astype(bf16),
        "w1_blk": _lhsT_blocks(w1q).astype(bf16),                # [DFF/P, P, D/P, P]
        "w2_chunk": _rhs_chunks(w2q, ncd).astype(bf16),          # [D/ncd, DFF/P, P, ncd]
        "bias_in": _per_part(g("in_proj_b")).astype(f32),        # [P, 3D/P]
        "b1_t": _per_part(g("ff1_b")).astype(f32),               # [P, DFF/P]
        "alpha_t": _per_part(g("alpha")).astype(f32),
        "ab1_t": _per_part((g("alpha") * g("ff1_b")).astype(f32)),
        "gamma_t": _per_part((1.0 / (g("beta") + np.float32(1e-9))).astype(f32)),
        "n2w_bc": _bcast_row(g("norm2_w")).astype(f32),          # [P, D]
        "b2_bc": _bcast_row(g("ff2_b")).astype(f32),
    }
    return arrays, ws1, ws2


# ---------------------------------------------------------------- device program

def build_program(nc, *, S, D, H, DFF, ws1, ws2):
    import concourse.mybir as mybir
    import concourse.tile as tile
    from concourse.bass import ts
    from concourse.bass_isa import ReduceOp
    from concourse.masks import make_identity

    dt = mybir.dt
    AF = mybir.ActivationFunctionType
    OP = mybir.AluOpType

    DH = D // H
    assert DH == P, "layout assumes head dim == 128"
    ST = S // P           # token tiles
    KD = D // P           # D contraction tiles
    KF = DFF // P         # DFF contraction tiles / ff1 out tiles
    NCD = min(512, D)     # fo chunk for out_proj/ff2 (psum-bank sized)
    NOD = D // NCD
    NCS = min(512, S)     # s chunk
    NOS = S // NCS
    MTG = ST // 2         # token tiles per out_proj group
    inv_sqrt_dh = float(1.0 / np.sqrt(DH))

    # ---- DRAM I/O ----
    src_d = nc.dram_tensor("src", [S, D], dt.float32, kind="ExternalInput")
    srcob_d = nc.dram_tensor("srcob", [S, D], dt.bfloat16, kind="ExternalInput")
    w_in_d = nc.dram_tensor("w_in_blk", [3 * KD, P, KD, P], dt.bfloat16, kind="ExternalInput")
    wo_d = nc.dram_tensor("wo_chunk", [NOD, KD, P, NCD], dt.bfloat16, kind="ExternalInput")
    w1_d = nc.dram_tensor("w1_blk", [KF, P, KD, P], dt.bfloat16, kind="ExternalInput")
    w2_d = nc.dram_tensor("w2_chunk", [NOD, KF, P, NCD], dt.bfloat16, kind="ExternalInput")
    bin_d = nc.dram_tensor("bias_in", [P, 3 * KD], dt.float32, kind="ExternalInput")
    b1_d = nc.dram_tensor("b1_t", [P, KF], dt.float32, kind="ExternalInput")
    alpha_d = nc.dram_tensor("alpha_t", [P, KF], dt.float32, kind="ExternalInput")
    ab1_d = nc.dram_tensor("ab1_t", [P, KF], dt.float32, kind="ExternalInput")
    gam_d = nc.dram_tensor("gamma_t", [P, KF], dt.float32, kind="ExternalInput")
    n2w_d = nc.dram_tensor("n2w_bc", [P, D], dt.float32, kind="ExternalInput")
    b2_d = nc.dram_tensor("b2_bc", [P, D], dt.float32, kind="ExternalInput")
    out_d = nc.dram_tensor("out", [S, D], dt.float32, kind="ExternalOutput")
    xb2_d = nc.dram_tensor("xb2_spill", [ST, P, D], dt.bfloat16)
    h2a_d = nc.dram_tensor("h2a_spill", [NSPLIT, P, S], dt.bfloat16)

    with tile.TileContext(nc) as tc:
        # ---------- persistent constants ----------
        cp = tc.alloc_tile_pool(name="consts", bufs=1)
        identf = cp.tile([P, P], dt.float32)
        make_identity(nc, identf)
        ones_1b = cp.tile([1, P], dt.bfloat16)
        nc.any.memset(ones_1b[:], 1.0)
        magic_col = cp.tile([P, 1], dt.float32)
        nc.any.memset(magic_col[:], MAGIC)
        bin_sb = cp.tile([P, 3 * KD], dt.float32)
        nc.sync.dma_start(out=bin_sb[:], in_=bin_d.ap())
        b1_sb = cp.tile([P, KF], dt.float32)
        nc.sync.dma_start(out=b1_sb[:], in_=b1_d.ap())
        alpha_sb = cp.tile([P, KF], dt.float32)
        nc.sync.dma_start(out=alpha_sb[:], in_=alpha_d.ap())
        ab1_sb = cp.tile([P, KF], dt.float32)
        nc.sync.dma_start(out=ab1_sb[:], in_=ab1_d.ap())
        gam_sb = cp.tile([P, KF], dt.float32)
        nc.sync.dma_start(out=gam_sb[:], in_=gam_d.ap())
        c2_tok = cp.tile([P, ST], dt.float32)   # filled in phase 4
        sc2_bc = cp.tile([P, S], dt.float32)    # filled in phase 4

        # ================= phase 1: rmsnorm1 (n1w folded) + DMA transpose =====
        xp = tc.alloc_tile_pool(name="x2T_pool", bufs=1)
        x2T = xp.tile([P, KD, S], dt.bfloat16)

        p1 = tc.alloc_tile_pool(name="p1", bufs=2)
        for mt in range(ST):
            xt = p1.tile([P, D], dt.float32, tag="xt", bufs=3)
            nc.scalar.dma_start(out=xt[:], in_=src_d.ap()[ts(mt, P), :])
            sq = p1.tile([P, D], dt.float32, tag="sq", bufs=2)
            ss = p1.tile([P, 1], dt.float32, tag="ss")
            nc.scalar.activation(sq[:], xt[:], AF.Square, accum_out=ss[:])
            ms = p1.tile([P, 1], dt.float32, tag="ms")
            nc.vector.tensor_scalar(ms[:], ss[:], 1.0 / D, EPS, op0=OP.mult, op1=OP.add)
            rt = p1.tile([P, 1], dt.float32, tag="rt")
            nc.scalar.activation(rt[:], ms[:], AF.Sqrt)
            rs = p1.tile([P, 1], dt.float32, tag="rs")
            nc.vector.reciprocal(rs[:], rt[:])
            x2 = p1.tile([P, D], dt.bfloat16, tag="x2", bufs=3)
            nc.scalar.activation(x2[:], xt[:], AF.Copy, scale=rs[:])
            nc.scalar.dma_start_transpose(out=x2T[:, :, ts(mt, P)], in_=x2[:])
        p1.release()

        # ================= phase 2: fused in_proj + attention =================
        op_ = tc.alloc_tile_pool(name="oT_pool", bufs=1, side="right")
        oT_all = op_.tile([P, KD, S], dt.bfloat16)
        wop = tc.alloc_tile_pool(name="wo_res", bufs=1, side="right")
        NORES = NOD // 2   # out_proj no-chunks kept resident in SBUF
        wo_sb = wop.tile([P, NORES, KD, NCD], dt.bfloat16)
        for no in range(NORES):
            for ko in range(KD):
                nc.scalar.dma_start(out=wo_sb[:, no, ko, :], in_=wo_d.ap()[no, ko])

        p2w = tc.alloc_tile_pool(name="p2w", bufs=2)
        p2 = tc.alloc_tile_pool(name="p2", bufs=2)
        p2a = tc.alloc_tile_pool(name="p2a", bufs=2, space="PSUM")
        p2b = tc.alloc_tile_pool(name="p2b", bufs=3, space="PSUM")

        def attn_tail(h, es, vT, expT):
            # denominator: partition-sum of es (already tt-summed) on GpSimd,
            # reciprocal on DVE; folded into the o^T eviction. One head late.
            for sc in range(NOS):
                esb = p2.tile([P, NCS], dt.float32, tag="esb", name=f"esb_{h}_{sc}")
                nc.gpsimd.partition_all_reduce(esb[:], es[sc][:], P, ReduceOp.add)
                rb = p2.tile([P, NCS], dt.float32, tag="rb", name=f"rb_{h}_{sc}")
                nc.vector.reciprocal(rb[:], esb[:])
                po = p2a.tile([P, NCS], dt.float32, tag="po", bufs=2, name=f"po_{h}_{sc}")
                for tt in range(ST):
                    nc.tensor.matmul(po[:], vT[:, tt, :], expT[:, tt, ts(sc, NCS)],
                                     start=(tt == 0), stop=(tt == ST - 1))
                nc.vector.tensor_tensor(oT_all[:, h, ts(sc, NCS)], po[:], rb[:], OP.mult)

        prev = None
        for h in range(H):
            qkv = []
            for j, mo in ((0, h), (1, KD + h), (2, 2 * KD + h)):
                wblk = p2w.tile([P, KD, P], dt.bfloat16, tag="wblk", bufs=3)
                nc.sync.dma_start(out=wblk[:], in_=w_in_d.ap()[mo])
                dest = p2.tile([P, S], dt.bfloat16, tag=f"qkv{j}", name=f"qkv{j}_{h}")
                for sc in range(NOS):
                    ps = p2a.tile([P, NCS], dt.float32, tag="mmps", bufs=3,
                                  name=f"qkvps_{h}_{j}_{sc}")
                    for ko in range(KD):
                        nc.tensor.matmul(ps[:], wblk[:, ko, :], x2T[:, ko, ts(sc, NCS)],
                                         start=(ko == 0), stop=(ko == KD - 1))
                    nc.scalar.activation(dest[:, ts(sc, NCS)], ps[:], AF.Identity,
                                         bias=bin_sb[:, mo:mo + 1])
                qkv.append(dest)
            q, k, v = qkv
            # scores^T -> exp; accumulate the tt-sum for the denominator on DVE
            expT = p2.tile([P, ST, S], dt.bfloat16, tag="expT", name=f"expT_{h}")
            es = [p2.tile([P, NCS], dt.bfloat16, tag=f"es{sc}", name=f"es_{h}_{sc}")
                  for sc in range(NOS)]
            for tt in range(ST):
                for sc in range(NOS):
                    ps = p2b.tile([P, NCS], dt.float32, tag="scps", bufs=3,
                                  name=f"scps_{h}_{tt}_{sc}")
                    nc.tensor.matmul(ps[:], k[:, ts(tt, P)], q[:, ts(sc, NCS)],
                                     start=True, stop=True)
                    nc.scalar.activation(expT[:, tt, ts(sc, NCS)], ps[:], AF.Exp,
                                         scale=inv_sqrt_dh)
                    if tt == 0:
                        nc.vector.tensor_copy(es[sc][:], expT[:, 0, ts(sc, NCS)])
                    else:
                        nc.vector.tensor_tensor(es[sc][:], es[sc][:],
                                                expT[:, tt, ts(sc, NCS)], OP.add)
            # v^T via DMA xbar transpose
            vT = p2.tile([P, ST, P], dt.bfloat16, tag="vT", name=f"vT_{h}")
            nc.scalar.dma_start_transpose(out=vT[:, :, :], in_=v[:])
            if prev is not None:
                attn_tail(*prev)
            prev = (h, es, vT, expT)
        attn_tail(*prev)
        p2b.release()
        p2a.release()
        p2.release()
        p2w.release()
        xp.release()  # x2T dead

        # ===== phase 3: out_proj + residual + rmsnorm2 + quant, 2 mt-groups =====
        qp = tc.alloc_tile_pool(name="x2qT_pool", bufs=1)
        x2qT = qp.tile([P, KD, S], dt.bfloat16)
        c1_bc = qp.tile([P, S], dt.float32)

        p3c = tc.alloc_tile_pool(name="p3c", bufs=1)
        p3x = tc.alloc_tile_pool(name="p3x", bufs=1)
        p3 = tc.alloc_tile_pool(name="p3", bufs=3)
        p3q = tc.alloc_tile_pool(name="p3q", bufs=2)
        p3ps = tc.alloc_tile_pool(name="p3ps", bufs=1, space="PSUM")
        n2w_sb = p3c.tile([P, D], dt.float32)
        nc.scalar.dma_start(out=n2w_sb[:], in_=n2w_d.ap())
        b2_sb = p3c.tile([P, D], dt.float32)
        nc.scalar.dma_start(out=b2_sb[:], in_=b2_d.ap())

        xg, ssl, c1c_of = {}, {}, {}

        def quant_chain(mt):
            xt = xg[mt]
            ss = p3q.tile([P, 1], dt.float32, tag="ss", bufs=4, name=f"ssq_{mt}")
            nc.vector.tensor_reduce(ss[:], ssl[mt][:], axis=mybir.AxisListType.X, op=OP.add)
            ms = p3q.tile([P, 1], dt.float32, tag="ms", bufs=4)
            nc.vector.tensor_scalar(ms[:], ss[:], 1.0 / D, EPS, op0=OP.mult, op1=OP.add)
            rt = p3q.tile([P, 1], dt.float32, tag="rt", bufs=4)
            nc.scalar.activation(rt[:], ms[:], AF.Sqrt)
            rs = p3q.tile([P, 1], dt.float32, tag="rs", bufs=4)
            nc.vector.reciprocal(rs[:], rt[:])
            x2 = p3q.tile([P, D], dt.float32, tag="x2", bufs=2)
            nc.vector.scalar_tensor_tensor(x2[:], xt[:], rs[:], n2w_sb[:],
                                           op0=OP.mult, op1=OP.mult)
            mx = p3q.tile([P, 1], dt.float32, tag="mx", bufs=4)
            nc.vector.tensor_reduce(mx[:], x2[:], axis=mybir.AxisListType.X,
                                    op=OP.max, apply_absolute_value=True)
            mcl = p3q.tile([P, 1], dt.float32, tag="mcl", bufs=4)
            nc.vector.tensor_scalar(mcl[:], mx[:], 1e-5, None, op0=OP.max)
            rc = p3q.tile([P, 1], dt.float32, tag="rc", bufs=4)
            nc.vector.reciprocal(rc[:], mcl[:])
            sc1 = p3q.tile([P, 1], dt.float32, tag="sc1", bufs=4)
            nc.vector.tensor_scalar(sc1[:], rc[:], 127.0, None, op0=OP.mult)
            c1c = p3q.tile([P, 1], dt.float32, tag="c1c", bufs=ST, name=f"c1c_{mt}")
            nc.vector.tensor_scalar(c1c[:], mcl[:], ws1 / 127.0, None, op0=OP.mult)
            c1c_of[mt] = c1c
            t1 = p3q.tile([P, D], dt.float32, tag="t1", bufs=2)
            nc.scalar.activation(t1[:], x2[:], AF.Identity, scale=sc1[:], bias=magic_col[:])
            xq = p3q.tile([P, D], dt.bfloat16, tag="xq", bufs=2)
            nc.vector.tensor_scalar(xq[:], t1[:], MAGIC, None, op0=OP.subtract)
            nc.scalar.dma_start_transpose(out=x2qT[:, :, ts(mt, P)], in_=xq[:])

        def bcast_col(col, dest_slice, trtag, bctag, pool, key):
            """col [P,1] f32 (partition=token) -> dest [P,128] f32 broadcast."""
            pt = pool.tile([1, P], dt.float32, tag=trtag, bufs=2, name=f"bt_{key}")
            nc.tensor.transpose(pt[:], col[:], identf[:])
            row = p3q.tile([1, P], dt.bfloat16, tag="brow", bufs=4, name=f"br_{key}")
            nc.scalar.activation(row[:], pt[:], AF.Copy)
            pbc = pool.tile([P, P], dt.float32, tag=bctag, bufs=2, name=f"bp_{key}")
            nc.tensor.matmul(pbc[:], ones_1b[:], row[:], start=True, stop=True)
            nc.vector.tensor_copy(dest_slice, pbc[:])

        def no_block(g, no, mts, psy):
            ch = ts(no, NCD)
            sds = {}
            for mt in mts:  # residual seed chunks (src + out_proj_b, bf16)
                sd = p3.tile([P, NCD], dt.bfloat16, tag="sd", bufs=8,
                             name=f"sd_{g}_{no}_{mt}")
                nc.scalar.dma_start(out=sd[:], in_=srcob_d.ap()[ts(mt, P), ch])
                sds[mt] = sd
            for ko in range(KD):
                if no < NORES:
                    wch = wo_sb[:, no, ko, :]
                else:
                    wt = p3.tile([P, NCD], dt.bfloat16, tag="wch", bufs=6)
                    nc.sync.dma_start(out=wt[:], in_=wo_d.ap()[no, ko])
                    wch = wt[:]
                for mt in mts:
                    nc.tensor.matmul(psy[mt][:], oT_all[:, ko, ts(mt, P)], wch,
                                     start=(ko == 0), stop=(ko == KD - 1))
            for mt in mts:
                nc.vector.tensor_tensor(xg[mt][:, ch], psy[mt][:], sds[mt][:], OP.add)
                if no < NOD - 1:
                    psy[mt] = p3ps.tile([P, NCD], dt.float32, tag=f"y{mt % MTG}",
                                        bufs=2, name=f"psy3_{g}_{mt}_{no + 1}")
            for mt in mts:
                xo = p3.tile([P, NCD], dt.bfloat16, tag="xo", bufs=4,
                             name=f"xo_{g}_{no}_{mt}")
                nc.vector.tensor_tensor(xo[:], xg[mt][:, ch], b2_sb[:, ch], OP.add)
                nc.scalar.dma_start(out=xb2_d.ap()[mt][:, ch], in_=xo[:])
                sqt = p3.tile([P, NCD], dt.float32, tag="sqt", bufs=4,
                              name=f"sqt_{g}_{no}_{mt}")
                nc.scalar.activation(sqt[:], xg[mt][:, ch], AF.Square,
                                     accum_out=ssl[mt][:, no:no + 1])

        def group_setup(g):
            mts = list(range(g * MTG, (g + 1) * MTG))
            for mt in mts:
                xg[mt] = p3x.tile([P, D], dt.bfloat16, tag=f"xg{mt % MTG}", bufs=1,
                                  name=f"xg_{mt}")
                ssl[mt] = p3q.tile([P, NOD], dt.float32, tag=f"ssl{mt % MTG}", bufs=2,
                                   name=f"ssl_{mt}")
            psy = {mt: p3ps.tile([P, NCD], dt.float32, tag=f"y{mt % MTG}", bufs=2,
                                 name=f"psy3_{g}_{mt}")
                   for mt in mts}
            return mts, psy

        mts0, psy0 = group_setup(0)
        for no in range(NOD):
            no_block(0, no, mts0, psy0)
        mts1, psy1 = group_setup(1)
        for no in range(NOD):
            no_block(1, no, mts1, psy1)
            quant_chain(mts0[no])       # group-0 chains hide under group-1 MMs
        for mt in mts0:                  # c1 broadcast, token half 0
            bcast_col(c1c_of[mt], c1_bc[:, ts(mt, P)], "y2", "y3", p3ps, f"c1_{mt}")
        p3ps.release()
        wop.release()
        op_.release()  # oT_all dead

        # ======= phase 4a: ff1 rows 0..NSPLIT-1 (h2 -> DRAM); group-1 chains ===
        p4 = tc.alloc_tile_pool(name="p4", bufs=2, side="right")
        p4m = tc.alloc_tile_pool(name="p4m", bufs=1, side="right")
        p4ps = tc.alloc_tile_pool(name="p4ps", bufs=3, space="PSUM")
        M_acc = p4m.tile([P, S], dt.float32)
        nc.any.memset(M_acc[:], 0.0)

        def ff1_unit(mo, sc, wblk, h2dest, h2spill):
            ph = p4ps.tile([P, NCS], dt.float32, tag="ph", name=f"ph_{mo}_{sc}")
            for ko in range(KD):
                nc.tensor.matmul(ph[:], wblk[:, ko, :], x2qT[:, ko, ts(sc, NCS)],
                                 start=(ko == 0), stop=(ko == KD - 1))
            ch = ts(sc, NCS)
            t_ = p4.tile([P, NCS], dt.float32, tag="t_", name=f"t_{mo}_{sc}")
            nc.vector.tensor_tensor(t_[:], ph[:], c1_bc[:, ch], OP.mult)
            s_ = p4.tile([P, NCS], dt.float32, tag="s_", name=f"s_{mo}_{sc}")
            nc.scalar.activation(s_[:], t_[:], AF.Sin,
                                 scale=alpha_sb[:, mo:mo + 1], bias=ab1_sb[:, mo:mo + 1])
            h_ = p4.tile([P, NCS], dt.float32, tag="h_", name=f"h_{mo}_{sc}")
            nc.scalar.activation(h_[:], t_[:], AF.Identity, bias=b1_sb[:, mo:mo + 1])
            sq_ = p4.tile([P, NCS], dt.float32, tag="sq_", name=f"sq_{mo}_{sc}")
            nc.scalar.activation(sq_[:], s_[:], AF.Square)
            nc.vector.scalar_tensor_tensor(h2dest, sq_[:], gam_sb[:, mo:mo + 1], h_[:],
                                           op0=OP.mult, op1=OP.add)
            if h2spill is not None:
                nc.scalar.dma_start(out=h2spill, in_=h2dest)
            am_ = p4.tile([P, NCS], dt.float32, tag="am_", name=f"am_{mo}_{sc}")
            nc.vector.scalar_tensor_tensor(am_[:], h2dest, -1.0, h2dest,
                                           op0=OP.mult, op1=OP.max)
            nc.vector.tensor_tensor(M_acc[:, ch], M_acc[:, ch], am_[:], OP.max)

        def w1_load(mo, key):
            wblk = p4.tile([P, KD, P], dt.bfloat16, tag="wblk", bufs=2, name=f"w1_{key}")
            nc.sync.dma_start(out=wblk[:], in_=w1_d.ap()[mo])
            return wblk

        def ff1_spill_unit(mo, sc, wblk):
            h2t = p4.tile([P, NCS], dt.bfloat16, tag="h2t", bufs=3,
                          name=f"h2t_{mo}_{sc}")
            ff1_unit(mo, sc, wblk, h2t[:], h2a_d.ap()[mo][:, ts(sc, NCS)])

        for mo in range(NSPLIT):   # token half 0; group-1 chains interleaved
            ff1_spill_unit(mo, 0, w1_load(mo, f"a{mo}"))
            if mo < MTG:
                quant_chain(mts1[mo])
        for mt in mts1:            # c1 broadcast, token half 1
            bcast_col(c1c_of[mt], c1_bc[:, ts(mt, P)], "pmt", "pb4", p4ps, f"c1_{mt}")
        for mo in range(NSPLIT):   # token half 1
            ff1_spill_unit(mo, 1, w1_load(mo, f"b{mo}"))
        p3q.release()
        p3.release()
        p3x.release()
        p3c.release()

        # ======= phase 4b: ff1 rows NSPLIT.. with h2 resident in SBUF =========
        hp = tc.alloc_tile_pool(name="h2_pool", bufs=1)
        h2sb = hp.tile([P, KF, S], dt.bfloat16)

        for mo in range(NSPLIT, KF):
            wblk = w1_load(mo, f"c{mo}")
            for sc in range(NOS):
                ff1_unit(mo, sc, wblk, h2sb[:, mo, ts(sc, NCS)], None)

        # cross-partition absmax via PE transpose + free-axis reduce
        m_tok = p4m.tile([P, ST], dt.float32)
        for c in range(ST):
            pmt = p4ps.tile([P, P], dt.float32, tag="pmt", bufs=2, name=f"pmt_{c}")
            nc.tensor.transpose(pmt[:], M_acc[:, ts(c, P)], identf[:])
            nc.vector.tensor_reduce(m_tok[:, c:c + 1], pmt[:], axis=mybir.AxisListType.X, op=OP.max)
        mcl2 = p4m.tile([P, ST], dt.float32)
        nc.vector.tensor_scalar(mcl2[:], m_tok[:], 1e-5, None, op0=OP.max)
        rc2 = p4m.tile([P, ST], dt.float32)
        nc.vector.reciprocal(rc2[:], mcl2[:])
        sc2_tok = p4m.tile([P, ST], dt.float32)
        nc.vector.tensor_scalar(sc2_tok[:], rc2[:], 127.0, None, op0=OP.mult)
        nc.vector.tensor_scalar(c2_tok[:], mcl2[:], ws2 / 127.0, None, op0=OP.mult)
        brow4 = tc.alloc_tile_pool(name="brow4", bufs=1, side="right")

        def bcast_col4(col, dest_slice, key):
            pt = p4ps.tile([1, P], dt.float32, tag="pmt", bufs=2, name=f"bt4_{key}")
            nc.tensor.transpose(pt[:], col[:], identf[:])
            row = brow4.tile([1, P], dt.bfloat16, tag="brow", bufs=4, name=f"br4_{key}")
            nc.scalar.activation(row[:], pt[:], AF.Copy)
            pbc = p4ps.tile([P, P], dt.float32, tag="pb4", bufs=2, name=f"bp4_{key}")
            nc.tensor.matmul(pbc[:], ones_1b[:], row[:], start=True, stop=True)
            nc.vector.tensor_copy(dest_slice, pbc[:])

        for mt in range(ST):  # per-token ff2 quant scale broadcast
            bcast_col4(sc2_tok[:, mt:mt + 1], sc2_bc[:, ts(mt, P)], f"s2_{mt}")
        brow4.release()
        p4ps.release()
        p4m.release()
        p4.release()

        # ===== phase 5: ff2; h2 quantized to ints in place (bf16) =====
        p6 = tc.alloc_tile_pool(name="p6", bufs=3, side="right")
        p6ps = tc.alloc_tile_pool(name="p6ps", bufs=1, space="PSUM")

        for ko in range(NSPLIT):  # read the spilled rows back into h2sb
            nc.scalar.dma_start(out=h2sb[:, ko, :], in_=h2a_d.ap()[ko])
        for ko in range(KF):
            src_ko = h2sb[:, ko, :]
            for sch in range(NOS):
                m1 = p6.tile([P, NCS], dt.float32, tag="m1", bufs=3,
                             name=f"m1_{ko}_{sch}")
                nc.vector.tensor_tensor(m1[:], src_ko[:, ts(sch, NCS)],
                                        sc2_bc[:, ts(sch, NCS)], OP.mult)
                nc.vector.tensor_scalar(src_ko[:, ts(sch, NCS)], m1[:], MAGIC, MAGIC,
                                        op0=OP.add, op1=OP.subtract)

        for no in range(NOD):
            psy = [p6ps.tile([P, NCD], dt.float32, tag=f"y{mt}", name=f"psy6_{no}_{mt}")
                   for mt in range(ST)]
            xchs = []
            for mt in range(ST):  # prefetch residual chunks (bf16)
                xch = p6.tile([P, NCD], dt.bfloat16, tag="xch", bufs=10,
                              name=f"xch6_{no}_{mt}")
                nc.scalar.dma_start(out=xch[:], in_=xb2_d.ap()[mt][:, ts(no, NCD)])
                xchs.append(xch)
            for ko in range(KF):
                wch = p6.tile([P, NCD], dt.bfloat16, tag="wch", bufs=12)
                nc.sync.dma_start(out=wch[:], in_=w2_d.ap()[no, ko])
                for mt in range(ST):
                    nc.tensor.matmul(psy[mt][:], h2sb[:, ko, ts(mt, P)], wch[:],
                                     start=(ko == 0), stop=(ko == KF - 1))
            for mt in range(ST):
                oe1 = p6.tile([P, NCD], dt.float32, tag="oe1", bufs=2, name=f"oe1_{no}_{mt}")
                nc.scalar.activation(oe1[:], psy[mt][:], AF.Identity,
                                     scale=c2_tok[:, mt:mt + 1])
                oe = p6.tile([P, NCD], dt.float32, tag="oe", bufs=2, name=f"oe_{no}_{mt}")
                nc.vector.tensor_tensor(oe[:], oe1[:], xchs[mt][:], OP.add)
                nc.scalar.dma_start(out=out_d.ap()[ts(mt, P), ts(no, NCD)], in_=oe[:])
        p6ps.release()
        p6.release()
        hp.release()
        qp.release()
        cp.release()
    return nc


# ---------------------------------------------------------------- driver

def _get_compiled(key, S, D, H, DFF, ws1, ws2):
    if key in _CACHE:
        return _CACHE[key]
    from concourse import bacc

    nc = bacc.Bacc("TRN2", target_bir_lowering=False, debug=False, num_devices=NCORES)
    build_program(nc, S=S, D=D, H=H, DFF=DFF, ws1=ws1, ws2=ws2)
    nc.compile()
    _CACHE[key] = nc
    return nc


def make_in_maps(inputs):
    src = np.asarray(inputs["src"], dtype=np.float32)
    B, S, D = src.shape
    H = H_FULL
    DFF = inputs["ff1_w"].shape[0]
    arrays, ws1, ws2 = _prep_arrays(inputs, S, D, H, DFF)
    import ml_dtypes
    srcob = (src + np.asarray(inputs["out_proj_b"], dtype=np.float32)[None, None, :]
             ).astype(ml_dtypes.bfloat16)
    in_maps = []
    for c in range(NCORES):
        m = dict(arrays)
        m["src"] = np.ascontiguousarray(src[c])
        m["srcob"] = np.ascontiguousarray(srcob[c])
        in_maps.append(m)
    return in_maps, (S, D, H, DFF, ws1, ws2)


def kernel(**inputs):
    from concourse.bass_utils import run_bass_kernel_spmd

    in_maps, (S, D, H, DFF, ws1, ws2) = make_in_maps(inputs)
    assert np.asarray(inputs["src"]).shape[0] == NCORES
    nc = _get_compiled(("full", S, D, H, DFF, ws1, ws2), S, D, H, DFF, ws1, ws2)
    res = run_bass_kernel_spmd(nc, in_maps, core_ids=list(range(NCORES)))
    out = np.stack([res.results[c]["out"] for c in range(NCORES)], axis=0)
    return out.astype(np.float32)
